# revision 8
# baseline (speedup 1.0000x reference)
"""MultiPool (segment_reduce) Trainium2 kernel.

Problem: embs [500000, 128] f32, batches [1048576] i32 (row indices),
lens = CSR pointer arange(65537)*16 (uniform segments of 16 rows).
Output [65536, 512] = concat([seg_sum, seg_mean, seg_min, seg_max], axis=1).

Strategy (pure data parallel over segments, embs replicated):
  - 8 cores, each owns 8192 consecutive segments (131072 gathered rows).
  - Host pre-permutes each core's indices to [128 partitions, 64 tiles * 16].
  - Per 128-segment supertile: one gpsimd indirect DMA gathers 2048 rows so
    that partition p holds the 16 rows of segment p contiguously
    ([128, 16, 128] f32). Segment reductions are then free-dim reductions:
    three DVE tensor_reduce ops (add/min/max) over the row axis, mean on the
    scalar engine (sum * 1/16), and one [128, 512] store per supertile.
"""

import os

import numpy as np

N_TABLE = 500_000
D = 128
B_SEGS = 65_536
SEG = 16
T_ROWS = B_SEGS * SEG
NCORES = 8
P = 128

SEGS_PER_CORE = B_SEGS // NCORES          # 8192
ROWS_PER_CORE = SEGS_PER_CORE * SEG       # 131072
TILES = SEGS_PER_CORE // P                # 64 supertiles of 128 segments

_cached_nc = None
last_results = None


def _build_program(n_table=N_TABLE, d=D, seg=SEG, tiles=TILES, g_bufs=4, o_bufs=4,
                   do_gather=True, do_reduce=True):
    from concourse import bass, bacc, mybir
    import concourse.tile as tile

    nc = bacc.Bacc(
        "TRN2",
        target_bir_lowering=False,
        debug=False,
        num_devices=NCORES,
        num_swdge_queues=4,
        dynamic_dma_scratch_size=65536,
    )
    embs = nc.dram_tensor("embs", [n_table, d], mybir.dt.float32, kind="ExternalInput").ap()
    idx = nc.dram_tensor("idx", [P, tiles * seg], mybir.dt.int32, kind="ExternalInput").ap()
    out = nc.dram_tensor("out", [tiles * P, 4 * d], mybir.dt.float32, kind="ExternalOutput").ap()

    with tile.TileContext(nc) as tc:
        with tc.tile_pool(name="idxp", bufs=1) as idxp, \
             tc.tile_pool(name="gp", bufs=g_bufs) as gp, \
             tc.tile_pool(name="op", bufs=o_bufs) as op:
            idx_sb = idxp.tile([P, tiles * seg], mybir.dt.int32, tag="idx")
            nc.sync.dma_start(out=idx_sb[:], in_=idx[:])
            for t in range(tiles):
                g = gp.tile([P, seg, d], mybir.dt.float32, tag="g")
                # HW contract: offset AP [P,1], dest [P,elem] — one gathered
                # row per partition per call. Row c of every segment in one go.
                if do_gather:
                    for c in range(seg):
                        col = t * seg + c
                        inst = nc.gpsimd.indirect_dma_start(
                            out=g[:, c, :],
                            out_offset=None,
                            in_=embs[:],
                            in_offset=bass.IndirectOffsetOnAxis(
                                ap=idx_sb[:, col:col + 1],
                                axis=0,
                            ),
                        )
                        # Round-robin the SWDGE queues: queue-serialization is
                        # ~1/3 of the per-call cost (HW A/B: 2 queues cut the
                        # marginal 2425us -> 1610us, bit-exact).
                        q = col % 4
                        if q:
                            inst.ins.queue = f"qPoolDynamic{q}"
                if not do_gather:
                    nc.scalar.memzero(g[:])
                if not do_reduce:
                    nc.sync.dma_start(out=out[t * P:(t + 1) * P, :],
                                      in_=g[:, 0:4, :].rearrange("p r d -> p (r d)"))
                    continue
                o = op.tile([P, 4 * d], mybir.dt.float32, tag="o")
                # View rows on the innermost axis: [P, d, seg] with the
                # row axis strided by d. tensor_reduce(X) reduces rows.
                gv = g[:].rearrange("p r d -> p d r")
                nc.vector.tensor_reduce(
                    out=o[:, 0:d], in_=gv,
                    axis=mybir.AxisListType.X, op=mybir.AluOpType.add)
                nc.vector.tensor_reduce(
                    out=o[:, 2 * d:3 * d], in_=gv,
                    axis=mybir.AxisListType.X, op=mybir.AluOpType.min)
                nc.vector.tensor_reduce(
                    out=o[:, 3 * d:4 * d], in_=gv,
                    axis=mybir.AxisListType.X, op=mybir.AluOpType.max)
                nc.scalar.mul(o[:, d:2 * d], o[:, 0:d], 1.0 / seg)
                nc.sync.dma_start(out=out[t * P:(t + 1) * P, :], in_=o[:])
    nc.compile()
    return nc


def _permute_indices(batches_core, tiles=TILES, seg=SEG):
    # [tiles*P*seg] -> [P, tiles*seg] so that partition p, tile t holds the
    # seg indices of segment (t*P + p) contiguously.
    return np.ascontiguousarray(
        batches_core.reshape(tiles, P, seg).transpose(1, 0, 2).reshape(P, tiles * seg)
    )


def kernel(embs, batches, lens=None):
    global _cached_nc, last_results
    from concourse import bass_utils

    embs = np.ascontiguousarray(np.asarray(embs), dtype=np.float32)
    batches = np.ascontiguousarray(np.asarray(batches), dtype=np.int32)

    if _cached_nc is None:
        _cached_nc = _build_program()
    nc = _cached_nc

    in_maps = []
    for c in range(NCORES):
        bc = batches[c * ROWS_PER_CORE:(c + 1) * ROWS_PER_CORE]
        in_maps.append({"embs": embs, "idx": _permute_indices(bc)})

    trace = bool(os.environ.get("KERNEL_TRACE"))
    res = bass_utils.run_bass_kernel_spmd(
        nc, in_maps, core_ids=list(range(NCORES)), trace=trace,
    )
    last_results = res
    return np.concatenate([res.results[c]["out"] for c in range(NCORES)], axis=0)


# revision 9
# speedup vs baseline: 1.3313x; 1.3313x over previous
"""MultiPool (segment_reduce) Trainium2 kernel.

Problem: embs [500000, 128] f32, batches [1048576] i32 (row indices),
lens = CSR pointer arange(65537)*16 (uniform segments of 16 rows).
Output [65536, 512] = concat([seg_sum, seg_mean, seg_min, seg_max], axis=1).

Strategy (pure data parallel over segments, embs replicated):
  - 8 cores, each owns 8192 consecutive segments (131072 gathered rows).
  - Host pre-permutes each core's indices to [128 partitions, 64 tiles * 16].
  - Per 128-segment supertile: one gpsimd indirect DMA gathers 2048 rows so
    that partition p holds the 16 rows of segment p contiguously
    ([128, 16, 128] f32). Segment reductions are then free-dim reductions:
    three DVE tensor_reduce ops (add/min/max) over the row axis, mean on the
    scalar engine (sum * 1/16), and one [128, 512] store per supertile.
"""

import os

import numpy as np

N_TABLE = 500_000
D = 128
B_SEGS = 65_536
SEG = 16
T_ROWS = B_SEGS * SEG
NCORES = 8
P = 128

SEGS_PER_CORE = B_SEGS // NCORES          # 8192
ROWS_PER_CORE = SEGS_PER_CORE * SEG       # 131072
TILES = SEGS_PER_CORE // P                # 64 supertiles of 128 segments

_cached_nc = None
last_results = None


def _build_program(n_table=N_TABLE, d=D, seg=SEG, tiles=TILES, g_bufs=4, o_bufs=4,
                   do_gather=True, do_reduce=True):
    from concourse import bass, bacc, mybir
    import concourse.tile as tile

    nc = bacc.Bacc(
        "TRN2",
        target_bir_lowering=False,
        debug=False,
        num_devices=NCORES,
        num_swdge_queues=4,
    )
    embs = nc.dram_tensor("embs", [n_table, d], mybir.dt.float32, kind="ExternalInput").ap()
    idx = nc.dram_tensor("idx", [P, tiles * seg], mybir.dt.int32, kind="ExternalInput").ap()
    out = nc.dram_tensor("out", [tiles * P, 4 * d], mybir.dt.float32, kind="ExternalOutput").ap()

    with tile.TileContext(nc) as tc:
        with tc.tile_pool(name="idxp", bufs=1) as idxp, \
             tc.tile_pool(name="gp", bufs=g_bufs) as gp, \
             tc.tile_pool(name="op", bufs=o_bufs) as op:
            idx_sb = idxp.tile([P, tiles * seg], mybir.dt.int32, tag="idx")
            nc.sync.dma_start(out=idx_sb[:], in_=idx[:])
            for t in range(tiles):
                g = gp.tile([P, seg, d], mybir.dt.float32, tag="g")
                # HW contract: offset AP [P,1], dest [P,elem] — one gathered
                # row per partition per call. Row c of every segment in one go.
                if do_gather:
                    for c in range(seg):
                        col = t * seg + c
                        inst = nc.gpsimd.indirect_dma_start(
                            out=g[:, c, :],
                            out_offset=None,
                            in_=embs[:],
                            in_offset=bass.IndirectOffsetOnAxis(
                                ap=idx_sb[:, col:col + 1],
                                axis=0,
                            ),
                        )
                        # Round-robin the SWDGE queues: queue-serialization is
                        # ~1/3 of the per-call cost (HW A/B: 2 queues cut the
                        # marginal 2425us -> 1610us, bit-exact).
                        q = col % 4
                        if q:
                            inst.ins.queue = f"qPoolDynamic{q}"
                if not do_gather:
                    nc.scalar.memzero(g[:])
                if not do_reduce:
                    nc.sync.dma_start(out=out[t * P:(t + 1) * P, :],
                                      in_=g[:, 0:4, :].rearrange("p r d -> p (r d)"))
                    continue
                o = op.tile([P, 4 * d], mybir.dt.float32, tag="o")
                # View rows on the innermost axis: [P, d, seg] with the
                # row axis strided by d. tensor_reduce(X) reduces rows.
                gv = g[:].rearrange("p r d -> p d r")
                nc.vector.tensor_reduce(
                    out=o[:, 0:d], in_=gv,
                    axis=mybir.AxisListType.X, op=mybir.AluOpType.add)
                nc.vector.tensor_reduce(
                    out=o[:, 2 * d:3 * d], in_=gv,
                    axis=mybir.AxisListType.X, op=mybir.AluOpType.min)
                nc.vector.tensor_reduce(
                    out=o[:, 3 * d:4 * d], in_=gv,
                    axis=mybir.AxisListType.X, op=mybir.AluOpType.max)
                nc.scalar.mul(o[:, d:2 * d], o[:, 0:d], 1.0 / seg)
                nc.sync.dma_start(out=out[t * P:(t + 1) * P, :], in_=o[:])
    nc.compile()
    return nc


def _permute_indices(batches_core, tiles=TILES, seg=SEG):
    # [tiles*P*seg] -> [P, tiles*seg] so that partition p, tile t holds the
    # seg indices of segment (t*P + p) contiguously.
    return np.ascontiguousarray(
        batches_core.reshape(tiles, P, seg).transpose(1, 0, 2).reshape(P, tiles * seg)
    )


def kernel(embs, batches, lens=None):
    global _cached_nc, last_results
    from concourse import bass_utils

    embs = np.ascontiguousarray(np.asarray(embs), dtype=np.float32)
    batches = np.ascontiguousarray(np.asarray(batches), dtype=np.int32)

    if _cached_nc is None:
        _cached_nc = _build_program()
    nc = _cached_nc

    in_maps = []
    for c in range(NCORES):
        bc = batches[c * ROWS_PER_CORE:(c + 1) * ROWS_PER_CORE]
        in_maps.append({"embs": embs, "idx": _permute_indices(bc)})

    trace = bool(os.environ.get("KERNEL_TRACE"))
    res = bass_utils.run_bass_kernel_spmd(
        nc, in_maps, core_ids=list(range(NCORES)), trace=trace,
    )
    last_results = res
    return np.concatenate([res.results[c]["out"] for c in range(NCORES)], axis=0)
